# revision 56
# baseline (speedup 1.0000x reference)
"""Trainium2 Bass kernel for nn_BatchDifferentiableKF (v5b — best measured).

Batched 4-state Kalman filter, B=16384 rows, T=512 steps, state
[px, py, vx, vy], measurements = predicted velocities (B, T, 2).

Structure exploited (weight-stationary/transposed formulation):
  * Gains are data-independent -> fixed schedule computed on host; the
    x/y channels decouple into two identical scalar filters LINEAR in
    (z, p0).
  * T is chunked [16, 62 x 8].  Per chunk the map (carry, z) -> outputs
    is one matmul:  out[4L, B] = lhsT[4+2L, 4L].T @ rhs[4+2L, B] where
    rhs = [p_c0 ; p_c1 ; v_c0 ; v_c1 ; z rows (4+2t+c)].  62-step
    chunks make the contract dim exactly 4+124=128 — carry is free.
  * Output rows are ordered so rows 0:4 of each chunk's output ARE the
    next chunk's carry rows: the "scan" is a [4, N] fp16 SBUF copy per
    chunk-group; no chain matmuls.
  * Weights are the STATIONARY operand (batch = moving free dim):
    LDWEIGHTS amortized over the batch, streams are N=512.
  * fp16 off-chip: host casts z (+p0) to fp16 time-major (4.7 MB/core
    read), outputs written fp16 time-major (8.4 MB/core) and upcast on
    host.  End-to-end scale-rel ~1.1e-3 (numpy-validated).

Performance notes (from v4-v12 traces):
  * Input DMAs must be 128-partition, base-0 HWDGE transfers: base-4 /
    partial-partition / SWDGE / multi-chunk-AP variants all collapse
    onto fewer SDMA engines or add ~2-3us sem latency.  z blocks are
    host-padded to 128 rows; all DMA goes on the sync ring in FIFO
    order (weights, z blocks 0..8, then outputs as stages complete).
  * PSUM evacuation is the wall (TRN2 errata: DVE/ACT fp32-PSUM reads
    at 1x): ~1.2us per [128,1024] copy, split across DVE+ACT.
  * The kernel pays ~6us fixed preamble and ~10us fixed drain-tail
    (Tile exit barrier + final DMA receipt).

Sharding: embarrassingly parallel over batch across the 8 cores.
"""

import numpy as np

B_FULL = 16384
T = 512
N_CORES = 8
B_CORE = B_FULL // N_CORES  # 2048
CHUNKS = [16] + [62] * 8    # sum = 512
NG = 2                      # batch column groups per core
GW = B_CORE // NG           # 1024 columns per group

ZP_ROWS = 9 * 128           # host-padded: 9 blocks x 128 rows
W_COLS = 248 + 64           # steady lhsT cols | chunk0 lhsT cols


# ----------------------------------------------------------------------------
# Host-side weight construction (float64)
# ----------------------------------------------------------------------------

def _gains(dt, q_pos, q_vel, r_vel, n):
    dt = float(np.float32(dt))
    r = float(np.float32(r_vel)) + float(np.float32(1e-6))
    qp = float(np.float32(q_pos))
    qv = float(np.float32(q_vel))
    Ppp, Ppv, Pvv = 1.0, 0.0, 1.0
    k_p = np.zeros(n)
    k_v = np.zeros(n)
    for t in range(n):
        Ppv_ = Ppv + dt * Pvv
        Ppp_ = Ppp + 2.0 * dt * Ppv + dt * dt * Pvv + qp
        Pvv_ = Pvv + qv
        S = Pvv_ + r
        k_p[t] = Ppv_ / S
        k_v[t] = Pvv_ / S
        Ppp = Ppp_ - k_p[t] * Ppv_
        Ppv = Ppv_ - k_p[t] * Pvv_
        Pvv = Pvv_ - k_v[t] * Pvv_
    return k_p, k_v


def _scalar_chunk_map(k_p, k_v, dt, t0, L):
    """Affine map (p_in, v_in, z_0..z_{L-1}) -> (p_i, v_i), i=0..L-1."""
    g = dt - k_p
    a = 1.0 - k_v
    Wp = np.zeros((L, L))
    Wv = np.zeros((L, L))
    Bpv = np.zeros(L)
    Bvv = np.zeros(L)
    pz = np.zeros(L)
    vz = np.zeros(L)
    pv = 0.0
    vv = 1.0
    for i in range(L):
        t = t0 + i
        pz = pz + g[t] * vz
        pv = pv + g[t] * vv
        pz[i] += k_p[t]
        vz = a[t] * vz
        vv = a[t] * vv
        vz[i] += k_v[t]
        Wp[i] = pz
        Wv[i] = vz
        Bpv[i] = pv
        Bvv[i] = vv
    return Wp, Wv, Bpv, Bvv


def _out_row_order(L):
    """Out-row order: p_end c0,c1 ; v_end c0,c1 ; pos (2i+c) i<L-1 ;
    vel (2i+c) i<L-1."""
    rows = [(0, L - 1, 0), (0, L - 1, 1), (1, L - 1, 0), (1, L - 1, 1)]
    for i in range(L - 1):
        rows.append((0, i, 0))
        rows.append((0, i, 1))
    for i in range(L - 1):
        rows.append((1, i, 0))
        rows.append((1, i, 1))
    return rows


def _build_lhsT(k_p, k_v, dt, t0, L):
    """Contract rows: 0:4 = carry (p c0, p c1, v c0, v c1); 4+2j+c = z."""
    Wp, Wv, Bpv, Bvv = _scalar_chunk_map(k_p, k_v, dt, t0, L)
    lhsT = np.zeros((2 * L + 4, 4 * L))
    for r, (pv, i, c) in enumerate(_out_row_order(L)):
        W = Wp if pv == 0 else Wv
        lhsT[4 + c:4 + 2 * L:2, r] = W[i]
        lhsT[c, r] = 1.0 if pv == 0 else 0.0
        lhsT[2 + c, r] = Bpv[i] if pv == 0 else Bvv[i]
    return lhsT


def build_weights(dt, q_pos, q_vel, r_vel):
    """wpack [128, 312] fp16: cols 0:248 steady lhsT (carry rows 0:4,
    z rows 4:128), cols 248:312 chunk-0 lhsT (rows 0:36)."""
    k_p, k_v = _gains(dt, q_pos, q_vel, r_vel, T)
    dtf = float(np.float32(dt))
    lhsT0 = _build_lhsT(k_p, k_v, dtf, 0, CHUNKS[0])           # [36, 64]
    lhsTss = _build_lhsT(k_p, k_v, dtf, CHUNKS[0], CHUNKS[1])  # [128, 248]
    wpack = np.zeros((128, W_COLS))
    wpack[:, 0:248] = lhsTss
    wpack[0:36, 248:312] = lhsT0
    return wpack.astype(np.float16)


def _row_maps():
    """Global out_tm row index for pos[t,c] and vel[t,c]."""
    pos_rows = np.zeros((T, 2), np.int64)
    vel_rows = np.zeros((T, 2), np.int64)
    base = 0
    t0 = 0
    for L in CHUNKS:
        for c in (0, 1):
            pos_rows[t0 + L - 1, c] = base + c
            vel_rows[t0 + L - 1, c] = base + 2 + c
        for i in range(L - 1):
            for c in (0, 1):
                pos_rows[t0 + i, c] = base + 4 + 2 * i + c
                vel_rows[t0 + i, c] = base + 4 + 2 * (L - 1) + 2 * i + c
        base += 4 * L
        t0 += L
    return pos_rows, vel_rows


# ----------------------------------------------------------------------------
# Bass kernel
# ----------------------------------------------------------------------------

def build_nc():
    """Bass program for one core processing B_CORE batch rows."""
    import concourse.bass as bass  # noqa: F401
    import concourse.tile as tile
    from concourse import bacc, mybir
    from contextlib import ExitStack

    f32 = mybir.dt.float32
    f16 = mybir.dt.float16

    b = B_CORE
    nc = bacc.Bacc("TRN2", target_bir_lowering=False, debug=False)

    z_pack = nc.dram_tensor("z_pack", [ZP_ROWS, b], f16,
                            kind="ExternalInput").ap()
    wpack_d = nc.dram_tensor("wpack", [128, W_COLS], f16,
                             kind="ExternalInput").ap()
    out_tm = nc.dram_tensor("out_tm", [4 * T, b], f16,
                            kind="ExternalOutput").ap()

    with tile.TileContext(nc) as tc, ExitStack() as ctx:
        const = ctx.enter_context(tc.tile_pool(name="const", bufs=1))
        ztp = ctx.enter_context(tc.tile_pool(name="ztp", bufs=1))
        stp = ctx.enter_context(tc.tile_pool(name="stp", bufs=1))
        psa = ctx.enter_context(tc.tile_pool(name="psa", bufs=2,
                                             space="PSUM"))
        psb = ctx.enter_context(tc.tile_pool(name="psb", bufs=2,
                                             space="PSUM"))

        # ---- constants + inputs: one HWDGE ring (sync); FIFO order =
        # weights, z blocks 0..8, then outputs as stages complete ----
        wsb = const.tile([128, W_COLS], f16, name="wsb", tag="wsb")
        warmw = const.tile([128, 512], f16, name="warmw", tag="warmw")
        nc.sync.dma_start(wsb[:], wpack_d)

        # zt block m: partitions 0:4 carry rows (block 0: p0 from HBM;
        # else written on-device from the previous chunk's stage rows
        # 0:4), 4:2L+4 z rows (128-row base-0 DMAs spread across all 16
        # SDMA engines; base-4/partial/SWDGE variants measured slower).
        zt = [ztp.tile([128, b], f16, name=f"zt_{m}", tag=f"zt{m}")
              for m in range(9)]
        for m in range(9):
            nc.sync.dma_start(zt[m][:], z_pack[128 * m:128 * (m + 1), :])

        # ---- PE warm-up on a memset tile (no DMA dependency) ----
        nc.vector.memset(warmw[:], 0.03125)
        warm_ps = psa.tile([128, GW], f32, tag="psA")
        for wi in range(10):
            nc.tensor.matmul(warm_ps[:, 0:512], warmw[:, 0:128], warmw[:],
                             start=(wi == 0), stop=(wi == 9))

        # ---- stage tiles (fully unrolled: no output-DMA back-pressure) --
        stA = [stp.tile([64 if m == 0 else 128, b], f16, name=f"stA_{m}",
                        tag=f"stA{m}") for m in range(9)]
        stB = [None] + [stp.tile([120, b], f16, name=f"stB_{m}",
                                 tag=f"stB{m}") for m in range(1, 9)]

        base = 0
        for m, L in enumerate(CHUNKS):
            R = 4 * L
            K = 2 * L + 4
            wA = wsb[0:36, 248:312] if m == 0 else wsb[:, 0:128]
            MA = min(R, 128)

            ps_g = []
            for g in range(NG):
                gsl = slice(GW * g, GW * (g + 1))
                ps = psa.tile([128, GW], f32, tag="psA")
                # matmul N is capped at 512 (one PSUM bank): two per tile
                for h in range(GW // 512):
                    hsl = slice(GW * g + 512 * h, GW * g + 512 * (h + 1))
                    nc.tensor.matmul(ps[0:MA, 512 * h:512 * (h + 1)], wA,
                                     zt[m][0:K, hsl], start=True,
                                     stop=True)
                ps_g.append(ps)
            for g in range(NG):
                gsl = slice(GW * g, GW * (g + 1))
                # evacuate + carry-copy; alternate engines for balance
                if g % 2 == 0:
                    nc.vector.tensor_copy(stA[m][:, gsl], ps_g[g][0:MA, :])
                    if m < 8:
                        nc.scalar.copy(zt[m + 1][0:4, gsl],
                                       stA[m][0:4, gsl])
                else:
                    nc.scalar.copy(stA[m][:, gsl], ps_g[g][0:MA, :])
                    if m < 8:
                        nc.vector.tensor_copy(zt[m + 1][0:4, gsl],
                                              stA[m][0:4, gsl])

            if m > 0:
                for g in range(NG):
                    gsl = slice(GW * g, GW * (g + 1))
                    ps = psb.tile([120, GW], f32, tag="psB")
                    for h in range(GW // 512):
                        hsl = slice(GW * g + 512 * h,
                                    GW * g + 512 * (h + 1))
                        nc.tensor.matmul(ps[:, 512 * h:512 * (h + 1)],
                                         wsb[:, 128:248], zt[m][0:K, hsl],
                                         start=True, stop=True)
                    # balance PSUM evacuation across DVE and ACT
                    if g % 2 == 0:
                        nc.scalar.copy(stB[m][:, gsl], ps[:])
                    else:
                        nc.vector.tensor_copy(stB[m][:, gsl], ps[:])

            nc.sync.dma_start(out_tm[base:base + MA, :], stA[m][:])
            if m > 0:
                nc.sync.dma_start(out_tm[base + 128:base + 248, :],
                                  stB[m][:])
            base += R

    nc.compile()
    return nc


# ----------------------------------------------------------------------------
# Host entry point
# ----------------------------------------------------------------------------

_CACHE = {}

# test-harness knobs (ignored in normal use)
PROFILE = False
LAST_RESULT = None


def _get_nc():
    if "nc" not in _CACHE:
        _CACHE["nc"] = build_nc()
    return _CACHE["nc"]


def kernel(pred_vel, dt, p0, q_pos, q_vel, r_vel):
    from concourse.bass_utils import run_bass_kernel_spmd

    z = np.asarray(pred_vel, dtype=np.float32)
    p0 = np.asarray(p0, dtype=np.float32)
    assert z.shape == (B_FULL, T, 2) and p0.shape == (B_FULL, 2)

    wpack = build_weights(dt, q_pos, q_vel, r_vel)
    nc = _get_nc()

    in_maps = []
    for i in range(N_CORES):
        sl = slice(i * B_CORE, (i + 1) * B_CORE)
        z_tm = z[sl].reshape(B_CORE, 2 * T).T.astype(np.float16)
        zp = np.zeros((9, 128, B_CORE), np.float16)
        zp[0, 0] = p0[sl, 0].astype(np.float16)
        zp[0, 1] = p0[sl, 1].astype(np.float16)
        zp[0, 4:36] = z_tm[0:32]
        for m in range(1, 9):
            zp[m, 4:128] = z_tm[32 + 124 * (m - 1):32 + 124 * m]
        in_maps.append({"z_pack": zp.reshape(ZP_ROWS, B_CORE),
                        "wpack": wpack})

    res = run_bass_kernel_spmd(nc, in_maps, core_ids=list(range(N_CORES)),
                               trace=PROFILE)
    global LAST_RESULT
    LAST_RESULT = res

    pos_rows, vel_rows = _row_maps()
    pos = np.empty((B_FULL, T, 2), np.float32)
    vel = np.empty((B_FULL, T, 2), np.float32)
    for i, r in enumerate(res.results):
        sl = slice(i * B_CORE, (i + 1) * B_CORE)
        o = r["out_tm"]  # [2048, B_CORE] fp16
        pos[sl] = o[pos_rows.reshape(-1)].T.reshape(
            B_CORE, T, 2).astype(np.float32)
        vel[sl] = o[vel_rows.reshape(-1)].T.reshape(
            B_CORE, T, 2).astype(np.float32)
    return pos, vel
